# revision 1
# baseline (speedup 1.0000x reference)
"""CSWin attention Bass/Trainium2 kernel (SPMD over 8 NeuronCores), v2.

Problem: nn_CSWinAttention. B=2, H=W=56, N=2 candidates, C=128 channels,
8 heads x d=16, vertical-stripe windows Hsp=56, Wsp=7 -> 16 windows of
L=784 tokens. Plus LePE-style depthwise-3x3 rpe on the value.

Sharding: each core owns 2 windows (core c -> batch c//4, window cols
[14*(c%4), 14*(c%4)+14)).

Key differences vs the fp32 baseline (357us -> ~143us):
  - All attention matmuls in bf16 (fp32 streams 2-4x slower on the PE);
    Q^T/K^T "even/odd" head layouts are built on the HOST (no device
    transposes at all), as are v_aug (with the ones-column) and the
    halo'd V^T for the rpe conv.
  - One head-set (4 heads) at a time so the AV accumulators need only
    2 PSUM banks, freeing 6 banks for a 3-deep S^T tile ring: the PE
    keeps a multi-unit runway, stays continuously busy, and holds its
    high p-state (idle gaps otherwise drop it to 1.2 GHz, 2x cost).
  - Per (head, key-chunk): 2 QK^T matmuls (512+272 query cols, one
    PSUM bank each), one exp over the joined [112,784] AP, 2 AV
    matmuls accumulating out^T + a ones-row (softmax denominators).
    Emission interleaves the previous chunk's AV wave between QK^T
    waves so the in-order PE queue never blocks on exp output.
  - exp is split across engines: ~60% on ACT (Exp activation), ~40% on
    DVE via a Schraudolph bf16 bit trick (int16(x*A+B) bitcast to
    bf16); pair masks (multiplicative, diagonal block) run on GpSimd.
  - Normalization: denominator rows broadcast within each 32-partition
    group by DVE stream_shuffle (no PE matmul, no tile-mode switch),
    fast reciprocal, multiply.
  - Attention output stays channel-major; sc and the device-computed
    rpe are DMA'd out separately and combined (transpose + add) on the
    host, eliminating all transpose-back PE work. The depthwise-3x3
    rpe conv runs on DVE/GpSimd, its ops drip-fed one per qc iteration
    so they never delay the exp stream.
"""

import numpy as np
import ml_dtypes

BF16 = ml_dtypes.bfloat16

B, Hh, Ww, Nc, Cc = 2, 56, 56, 2, 128
HEADS, Dh, WSP = 8, 16, 7
L = Hh * WSP * Nc          # 784 tokens per window
PCH = 112                  # key-chunk (partition) size; 7 chunks
QC = L // PCH              # 7
H0, H1 = 512, 272          # query-dim halves for AV psum banks
SCALE = float(Dh) ** -0.5

# Schraudolph exp for bf16 bit pattern: bits = x*A7 + B7, bitcast int16->bf16
A7 = SCALE * 128.0 / float(np.log(2.0))
B7 = 127.0 * 128.0 - 7.42
# unit index (0..7) within each qc: which exps run on DVE instead of ACT,
# and which diag-masks run on GpSimd instead of DVE
EXP_DVE_UNITS = frozenset((1, 3, 5))
MASK_GP_UNITS = frozenset(range(8))

_cache = {}


def _build_program():
    import concourse.bacc as bacc
    import concourse.tile as tile
    from concourse import mybir

    f32 = mybir.dt.float32
    bf16 = mybir.dt.bfloat16
    i16 = mybir.dt.int16
    AT = mybir.AluOpType
    AF = mybir.ActivationFunctionType

    nc = bacc.Bacc("TRN2", target_bir_lowering=False, debug=False, num_devices=8)

    qtev_d = nc.dram_tensor("qtev", [Cc, 2, L], bf16, kind="ExternalInput")
    qtod_d = nc.dram_tensor("qtod", [PCH, 2, L], bf16, kind="ExternalInput")
    ktev_d = nc.dram_tensor("ktev", [Cc, 2, L], bf16, kind="ExternalInput")
    ktod_d = nc.dram_tensor("ktod", [PCH, 2, L], bf16, kind="ExternalInput")
    vaug_d = nc.dram_tensor("vaug", [PCH, 2, QC, HEADS, 24], bf16,
                            kind="ExternalInput")
    vtext_d = nc.dram_tensor("vtext", [Cc, 2, 8, 7, 9, 2], f32,
                             kind="ExternalInput")
    tapw_d = nc.dram_tensor("tapw", [Cc, 9], f32, kind="ExternalInput")
    cneg_d = nc.dram_tensor("cneg", [Cc, 1], f32, kind="ExternalInput")
    cpos_d = nc.dram_tensor("cpos", [Cc, 1], f32, kind="ExternalInput")
    maskd_d = nc.dram_tensor("maskd", [PCH, 2 * PCH], bf16, kind="ExternalInput")
    sc_d = nc.dram_tensor("sc", [Cc, 2, 2, L], f32, kind="ExternalOutput")
    rpe_d = nc.dram_tensor("rpe", [Cc, 2, L], f32, kind="ExternalOutput")

    # head pairs: (slot, heads, parity); parity 0 -> ev tiles, 1 -> od tiles
    PAIRS = [((0, 2), 0), ((4, 6), 0), ((1, 3), 1), ((5, 7), 1)]

    def rowbase(h, par):
        return 16 * h if par == 0 else 16 * (h - 1)

    with tile.TileContext(nc) as tc:
        with (
            tc.tile_pool(name="consts", bufs=1) as consts,
            tc.tile_pool(name="io", bufs=2) as io,
            tc.tile_pool(name="rpe", bufs=2) as rpep,
            tc.tile_pool(name="pt", bufs=10) as ptp,
            tc.tile_pool(name="post", bufs=4) as postp,
            tc.tile_pool(name="ps_st", bufs=3, space="PSUM") as ps_st,
            tc.tile_pool(name="ps_av", bufs=2, space="PSUM") as ps_av,
        ):
            tapw = consts.tile([Cc, 9], f32)
            nc.sync.dma_start(out=tapw[:], in_=tapw_d[:])
            cneg = consts.tile([Cc, 1], f32)
            nc.sync.dma_start(out=cneg[:], in_=cneg_d[:])
            cpos = consts.tile([Cc, 1], f32)
            nc.sync.dma_start(out=cpos[:], in_=cpos_d[:])
            maskd = consts.tile([PCH, 2 * PCH], bf16)
            nc.sync.dma_start(out=maskd[:], in_=maskd_d[:])

            for jj in range(2):
                # ---------- loads ----------
                qt_ev = io.tile([Cc, L], bf16, tag="qtev")
                nc.sync.dma_start(out=qt_ev[:], in_=qtev_d[:, jj, :])
                qt_od = io.tile([PCH, L], bf16, tag="qtod")
                nc.sync.dma_start(out=qt_od[:], in_=qtod_d[:, jj, :])
                kt_ev = io.tile([Cc, L], bf16, tag="ktev")
                nc.sync.dma_start(out=kt_ev[:], in_=ktev_d[:, jj, :])
                kt_od = io.tile([PCH, L], bf16, tag="ktod")
                nc.sync.dma_start(out=kt_od[:], in_=ktod_d[:, jj, :])
                v_aug = io.tile([PCH, QC, HEADS, 24], bf16, tag="vaug")
                nc.sync.dma_start(out=v_aug[:], in_=vaug_d[:, jj])
                vt_ext = io.tile([Cc, 8, 7, 9, 2], f32, tag="vtext")
                nc.sync.dma_start(out=vt_ext[:], in_=vtext_d[:, jj])

                # ---- rpe pre-ops (GpSimd) + deferred DVE op list ----
                vs_pad = rpep.tile([Cc, 58, 9], f32, tag="vs_pad")
                nc.gpsimd.memset(vs_pad[:], 0.0)
                nc.gpsimd.tensor_tensor(
                    vs_pad[:, 1:57, :].rearrange("c (yb y) x -> c yb y x", y=7),
                    vt_ext[:, :, :, :, 0],
                    vt_ext[:, :, :, :, 1],
                    AT.add,
                )
                conv_a = rpep.tile([Cc, 56, 7], f32, tag="conv_a")
                conv_b = rpep.tile([Cc, 56, 7], f32, tag="conv_b")
                cvs = rpep.tile([Cc, 56, 7], f32, tag="cvs")
                rpe = rpep.tile([Cc, 56, 7, 2], f32, tag="rpe")
                rpe_ops = []

                def _tap(t, acc_src, dst):
                    ky, kx = t // 3, t % 3
                    shifted = vs_pad[:, ky:ky + 56, kx:kx + 7]
                    if t == 0:
                        nc.vector.tensor_scalar(
                            dst[:], shifted, tapw[:, t:t + 1], None, AT.mult
                        )
                    else:
                        nc.vector.scalar_tensor_tensor(
                            dst[:], shifted, tapw[:, t:t + 1], acc_src[:],
                            AT.mult, AT.add,
                        )

                acc = None
                for t in range(9):
                    dst = conv_a if t % 2 == 0 else conv_b
                    rpe_ops.append(lambda t=t, a=acc, d=dst: _tap(t, a, d))
                    acc = dst
                rpe_ops.append(lambda a=acc: nc.vector.scalar_tensor_tensor(
                    cvs[:], vs_pad[:, 1:57, 1:8], cneg[:], a[:],
                    AT.mult, AT.add,
                ))
                for n in range(2):
                    rpe_ops.append(lambda n=n: nc.vector.scalar_tensor_tensor(
                        rpe[:, :, :, n],
                        vt_ext[:, :, :, 1:8, n].rearrange(
                            "c yb y x -> c (yb y) x"),
                        cpos[:],
                        cvs[:],
                        AT.mult, AT.add,
                    ))
                rpe_ops.append(lambda: nc.sync.dma_start(
                    out=rpe_d[:, jj, :],
                    in_=rpe[:].rearrange("c y x n -> c (y x n)"),
                ))
                rpe_iter = iter(rpe_ops)

                # ---------- attention: one head-set (4 heads) at a time ----
                for s in range(2):
                    # pairs within the set: (even heads), (odd heads)
                    SPAIRS = [((4 * s, 4 * s + 2), 0), ((4 * s + 1, 4 * s + 3), 1)]
                    av0 = ps_av.tile([Cc, H0], f32, tag="av",
                                     name=f"av0_{jj}_{s}")
                    av1 = ps_av.tile([Cc, H0], f32, tag="av",
                                     name=f"av1_{jj}_{s}")

                    def emit_av_head(qc, h, pt):
                        j = h % 4
                        lhsT = v_aug[:, qc, h, 0:Dh + 1]
                        nc.tensor.matmul(
                            av0[32 * j:32 * j + Dh + 1, 0:H0],
                            lhsT, pt[:, 0:H0],
                            start=(qc == 0), stop=(qc == QC - 1),
                            tile_position=(0, 32 * j),
                            skip_group_check=True,
                        )
                        nc.tensor.matmul(
                            av1[32 * j:32 * j + Dh + 1, 0:H1],
                            lhsT, pt[:, H0:L],
                            start=(qc == 0), stop=(qc == QC - 1),
                            tile_position=(0, 32 * j),
                            skip_group_check=True,
                        )

                    prev_pt = None
                    for qc in range(QC):
                        cur_pt = {}
                        for pi, (heads, par) in enumerate(SPAIRS):
                            kt, qt = ((kt_ev, qt_ev) if par == 0
                                      else (kt_od, qt_od))
                            sts = []
                            for h in heads:
                                base = rowbase(h, par)
                                st = ps_st.tile([PCH, 1024], f32, tag="st")
                                kts = kt[base:base + Dh,
                                         PCH * qc:PCH * (qc + 1)]
                                nc.tensor.matmul(
                                    st[:, 0:H0], kts,
                                    qt[base:base + Dh, 0:H0],
                                    start=True, stop=True,
                                    tile_position=(base, 0),
                                    skip_group_check=True,
                                )
                                nc.tensor.matmul(
                                    st[:, H0:L], kts,
                                    qt[base:base + Dh, H0:L],
                                    start=True, stop=True,
                                    tile_position=(base, 0),
                                    skip_group_check=True,
                                )
                                sts.append(st)
                            if prev_pt is not None:
                                for ui, h in enumerate(heads):
                                    emit_av_head(qc - 1, h,
                                                 prev_pt[2 * pi + ui])
                            for ui, h in enumerate(heads):
                                u = 2 * pi + ui
                                st = sts[ui]
                                pt = ptp.tile([PCH, L], bf16, tag="pt")
                                if (4 * qc + u) % 8 in EXP_DVE_UNITS:
                                    nc.vector.tensor_scalar(
                                        pt[:].bitcast(i16), st[:, 0:L],
                                        A7, B7, AT.mult, AT.add,
                                    )
                                else:
                                    nc.scalar.activation(
                                        pt[:], st[:, 0:L], AF.Exp,
                                        scale=SCALE,
                                    )
                                blk = pt[:, PCH * qc:PCH * (qc + 1)]
                                nc.gpsimd.tensor_tensor(
                                    blk, blk, maskd[:, 0:PCH], AT.mult)
                                cur_pt[u] = pt
                        prev_pt = cur_pt
                        op = next(rpe_iter, None)
                        if op is not None:
                            op()
                    for u in range(4):
                        h = 4 * s + [0, 2, 1, 3][u]
                        emit_av_head(QC - 1, h, prev_pt[u])

                    # ---------- normalize + store for this set ----------
                    # denominator rows (32j+16) broadcast within each
                    # 32-partition group via DVE stream_shuffle -- no PE
                    # matmul, no (128,128) tile-mode switch
                    av_sb = postp.tile([Cc, L], f32, tag="av_sb")
                    nc.vector.tensor_copy(av_sb[:, 0:H0], av0[:, 0:H0])
                    nc.vector.tensor_copy(av_sb[:, H0:L], av1[:, 0:H1])
                    den = postp.tile([Cc, L], f32, tag="den")
                    nc.vector.stream_shuffle(den[:], av_sb[:], [16] * 32)
                    drec = postp.tile([Cc, L], f32, tag="drec")
                    nc.vector.reciprocal_approx_fast(drec[:], den[:])
                    sc = postp.tile([Cc, L], f32, tag="sc")
                    nc.vector.tensor_tensor(sc[:], av_sb[:], drec[:], AT.mult)
                    nc.sync.dma_start(out=sc_d[:, jj, s, :], in_=sc[:])
                for op in rpe_iter:
                    op()

    nc.compile()
    return nc


def _host_inputs(query, key, value, conv_w):
    """Build the 8 per-core input dicts (layouts pre-transposed on host)."""
    query = np.ascontiguousarray(query, dtype=np.float32)
    key = np.ascontiguousarray(key, dtype=np.float32)
    value = np.ascontiguousarray(value, dtype=np.float32)
    conv_w = np.asarray(conv_w, dtype=np.float32)

    tapw = conv_w[:, 0].reshape(Cc, 9).copy()
    center = conv_w[:, 0, 1, 1].reshape(Cc, 1).copy()
    cneg = np.ascontiguousarray(-center)
    maskblk = np.ones((PCH, PCH), np.float32)
    idx = np.arange(PCH)
    maskblk[idx, idx ^ 1] = 0.0
    maskd = np.tile(maskblk, (1, 2)).astype(BF16)

    in_maps = []
    for c in range(8):
        b, jblk = c // 4, c % 4
        xs = 14 * jblk
        # halo'd value slice for rpe: x in [xs-1, xs+15)
        v_sl = np.zeros((Hh, 16, Nc, Cc), np.float32)
        v_sl[:, 1:15] = value[b, :, xs:xs + 14]
        if xs - 1 >= 0:
            v_sl[:, 0] = value[b, :, xs - 1]
        if xs + 14 < Ww:
            v_sl[:, 15] = value[b, :, xs + 14]

        qtev = np.empty((Cc, 2, L), BF16)
        qtod = np.empty((PCH, 2, L), BF16)
        ktev = np.empty((Cc, 2, L), BF16)
        ktod = np.empty((PCH, 2, L), BF16)
        vaug = np.zeros((PCH, 2, QC, HEADS, 24), BF16)
        vtext = np.empty((Cc, 2, 8, 7, 9, 2), np.float32)
        for jj in range(2):
            x0 = xs + WSP * jj
            for T, ev, od in ((query, qtev, qtod), (key, ktev, ktod)):
                t = T[b, :, x0:x0 + WSP].reshape(L, Cc).T  # [128, 784]
                ev[:, jj, :] = t
                od[:, jj, :] = t[Dh:Cc]
            va = value[b, :, x0:x0 + WSP].reshape(L, HEADS, Dh)
            vaug[:, jj, :, :, 0:Dh] = (
                va.reshape(QC, PCH, HEADS, Dh).transpose(1, 0, 2, 3))
            vaug[:, jj, :, :, Dh] = 1.0
            # vt_ext[c, b8, y_loc, x, n] with x halo (9 wide)
            vw = v_sl[:, 7 * jj:7 * jj + 9]  # [56, 9, 2, 128]
            vtext[:, jj] = vw.reshape(8, 7, 9, 2, Cc).transpose(4, 0, 1, 2, 3)

        in_maps.append({
            "qtev": qtev, "qtod": qtod, "ktev": ktev, "ktod": ktod,
            "vaug": vaug, "vtext": vtext,
            "tapw": tapw, "cneg": cneg, "cpos": center,
            "maskd": maskd,
        })
    return in_maps


def _run(in_maps, trace=False):
    from concourse.bass_utils import run_bass_kernel_spmd

    if "nc" not in _cache:
        _cache["nc"] = _build_program()
    return run_bass_kernel_spmd(
        _cache["nc"], in_maps, core_ids=list(range(8)), trace=trace
    )


def _assemble(res):
    out = np.zeros((B, Hh, Ww, Nc, Cc), np.float32)
    for c in range(8):
        b, jblk = c // 4, c % 4
        sc = np.asarray(res.results[c]["sc"], np.float32)   # [128, 2, 2, 784]
        rpe = np.asarray(res.results[c]["rpe"], np.float32)  # [128, 2, 784]
        for jj in range(2):
            att = np.empty((L, Cc), np.float32)
            for s in range(2):
                scw = sc[:, jj, s]  # [128, 784]
                for j in range(4):
                    att[:, 64 * s + 16 * j:64 * s + 16 * j + Dh] = (
                        scw[32 * j:32 * j + Dh].T)
            att += rpe[:, jj].T
            x0 = 14 * jblk + WSP * jj
            out[b, :, x0:x0 + WSP] = att.reshape(Hh, WSP, Nc, Cc)
    return out


def kernel(query, key, value, conv_w):
    in_maps = _host_inputs(query, key, value, conv_w)
    res = _run(in_maps)
    return _assemble(res)

